# revision 48
# baseline (speedup 1.0000x reference)
"""Trainium2 Bass kernel for nn_BboxEncoder (12-layer GPT-style transformer).

Strategy: data-parallel over batch (16 seqs -> 2 per core x 8 cores), no
collectives.  Activations kept TRANSPOSED [E, tokens] in SBUF; weights are
column-tiled bf16 so every GEMM has a [128,128] bf16 stationary operand
(FWL weight loads) and a 512-token moving operand.

LayerNorm gains/biases and the attention scale are folded into weights
host-side; the V-projection bias is folded through Wo into the O-projection
bias.  LN stats use PE ones-matmuls; rstd comes from one ACT Rsqrt.

Causal softmax: S^T = K @ Q^T per (seq, head) with triangular block
skipping, exp without max-subtraction, a 0/1 mask on the diagonal block.
Per-head column sums accumulate into one [16, T] PSUM tile via one-hot
stationary columns; a single reciprocal_approx_fast + per-2-head broadcast
normalizes the attention outputs.

Phases are emitted per sequence-half (ln -> V/Q/K -> attn -> O -> ln -> MLP)
so the Tile scheduler overlaps each half's vector/scalar chains with the
other half's GEMMs and the PE never idles long enough to re-throttle.
"""

import numpy as np
import ml_dtypes

import concourse.bass as bass
import concourse.mybir as mybir
import concourse.tile as tile
from concourse import bacc
from concourse.bass_utils import run_bass_kernel_spmd

P = 128
F32 = mybir.dt.float32
F32R = mybir.dt.float32r
BF16 = mybir.dt.bfloat16
AF = mybir.ActivationFunctionType
ADD = mybir.AluOpType.add
NCORES = 8
LN_EPS = 1e-5


class Cfg:
    def __init__(self, L=12, E=1024, H=16, T=512, NS=2, DIN=4):
        self.L, self.E, self.H, self.T, self.NS, self.DIN = L, E, H, T, NS, DIN
        self.HD = E // H            # head dim (64)
        self.EB = E // P            # E blocks (8)
        self.NB = T // P            # token blocks per seq (4)
        self.TOK = NS * T           # tokens per core (1024)
        self.TB = self.TOK // P     # token blocks per core (8)
        self.HID = 4 * E            # MLP hidden (4096)
        self.HIDB = self.HID // P   # hidden blocks (32)
        self.HPB = P // self.HD     # heads per 128-row block (2)
        self.EH = min(E, 512)       # E-half width for V gemm
        self.NEH = E // self.EH
        assert self.HD <= P and E % P == 0 and T % P == 0


def r32(ap):
    return ap.bitcast(F32R)


def tl(pool, shape, dt, tag):
    return pool.tile(shape, dt, tag=tag, name=tag)


def build_nc(cfg: Cfg, debug=False):
    c = cfg
    nc = bacc.Bacc(None)
    L, E, EB, NB, NS, TOK, TB = c.L, c.E, c.EB, c.NB, c.NS, c.TOK, c.TB
    HIDB, HD, HPB, T = c.HIDB, c.HD, c.HPB, c.T

    dp = nc.declare_dram_parameter
    xt_d = dp("xt", [c.DIN, TOK], F32, isOutput=False)
    we1_d = dp("wemb1", [c.DIN, E], F32, isOutput=False)
    we2_d = dp("wemb2col", [EB, P, EB, P], BF16, isOutput=False)
    whd_d = dp("wheadcol", [EB, P, EB, P], BF16, isOutput=False)
    wq_d = dp("wqcol", [L, EB, P, EB, P], BF16, isOutput=False)
    wk_d = dp("wkcol", [L, EB, P, EB, P], BF16, isOutput=False)
    wv_d = dp("wvrow", [L, EB, P, E], BF16, isOutput=False)
    wo_d = dp("wocol", [L, EB, P, EB, P], BF16, isOutput=False)
    w1_d = dp("w1col", [L, HIDB, P, EB, P], BF16, isOutput=False)
    w2_d = dp("w2col", [L, EB, P, HIDB, P], BF16, isOutput=False)
    bq_d = dp("bq_t", [L, P, EB], F32, isOutput=False)
    bk_d = dp("bk_t", [L, P, EB], F32, isOutput=False)
    bo_d = dp("bo_t", [L, P, EB], F32, isOutput=False)
    b1_d = dp("b1_t", [L, P, HIDB], F32, isOutput=False)
    b2_d = dp("b2_t", [L, P, EB], F32, isOutput=False)
    be1_d = dp("bemb1_t", [P, EB], F32, isOutput=False)
    be2_d = dp("bemb2_t", [P, EB], F32, isOutput=False)
    bh_d = dp("bhead_t", [P, EB], F32, isOutput=False)
    onc_d = dp("ones_col", [P, 1], F32, isOutput=False)
    onr_d = dp("ones_row", [1, P], BF16, isOutput=False)
    hsel_d = dp("hsel", [P, c.H, c.H], BF16, isOutput=False)
    hb2_d = dp("hb2", [HPB, P], BF16, isOutput=False)
    mbias_d = dp("mbias", [P, P], BF16, isOutput=False)
    id128_d = dp("id128", [P, P], BF16, isOutput=False)
    eps_d = dp("epsv", [1, 1], F32, isOutput=False)
    out_d = dp("out", [EB, P, TOK], F32, isOutput=True)
    dbg_d = dp("dbg", [8, P, T], F32, isOutput=True) if debug else None

    with tile.TileContext(nc) as tc:
        with (
            tc.tile_pool(name="st", bufs=1) as st,
            tc.tile_pool(name="wkp", bufs=1) as wkp,
            tc.tile_pool(name="ps", bufs=1, space="PSUM") as ps,
        ):
            # ---- constants ----
            onc = tl(st, [P, 1], F32R, "onc")
            nc.sync.dma_start(out=onc, in_=r32(onc_d[:, :]))
            onr = tl(st, [1, P], BF16, "onr")
            nc.sync.dma_start(out=onr, in_=onr_d[:, :])
            hsel = tl(st, [P, c.H, c.H], BF16, "hsel")
            nc.sync.dma_start(out=hsel, in_=hsel_d[:, :, :])
            hb2 = tl(st, [HPB, P], BF16, "hb2")
            nc.sync.dma_start(out=hb2, in_=hb2_d[:, :])
            mbias = tl(st, [P, P], BF16, "mbias")
            nc.sync.dma_start(out=mbias, in_=mbias_d[:, :])
            id128 = tl(st, [P, P], BF16, "id128")
            nc.sync.dma_start(out=id128, in_=id128_d[:, :])
            epst = tl(st, [1, 1], F32, "epst")
            nc.sync.dma_start(out=epst, in_=eps_d[:, :])

            # ---- persistent activations ----
            HT = [tl(st, [P, TOK], F32R, f"ht{i}") for i in range(EB)]
            AT = [tl(st, [P, TOK], BF16, f"at{i}") for i in range(EB)]
            QT = [tl(st, [P, T], BF16, f"qt{i}") for i in range(EB)]
            KT = [tl(st, [P, T], BF16, f"kt{i}") for i in range(EB)]
            VT = [[tl(st, [P, E], BF16, f"vt{s}_{i}") for i in range(NB)]
                  for s in range(NS)]
            OT = [[tl(st, [P, T], BF16, f"ot{s}_{i}") for i in range(EB)]
                  for s in range(NS)]
            GT = [tl(st, [P, T], BF16, f"gt{i}") for i in range(HIDB)]
            PT = [tl(st, [P, T], BF16, f"pt{i}") for i in range(NB)]

            def ln_half(s, dst, t0, t1):
                """dst[i][:, s-half] <- (HT - mu) * rstd for tokens of seq s.

                Uses PSUM tags t0/t1 for the stats accumulation, then reuses
                them for the rstd / -mu*rstd broadcasts (gain/bias folded
                into the consuming weights host-side)."""
                ssl = slice(s * T, (s + 1) * T)
                sum_ps = tl(ps, [1, T], F32, t0)
                sq_ps = tl(ps, [1, T], F32, t1)
                for t in range(EB):
                    sq = tl(wkp, [P, T], F32R, f"sq{t % 2}")
                    nc.vector.tensor_mul(sq[:, :], HT[t][:, ssl], HT[t][:, ssl])
                    nc.tensor.matmul(sum_ps[:, :], onc[:, :], HT[t][:, ssl],
                                     start=(t == 0), stop=(t == EB - 1))
                    nc.tensor.matmul(sq_ps[:, :], onc[:, :], sq[:, :],
                                     start=(t == 0), stop=(t == EB - 1))
                nmu = tl(wkp, [1, T], F32R, "lnm")
                nc.scalar.mul(nmu[:, :], sum_ps[:, :], -1.0 / E)
                msq = tl(wkp, [1, T], F32, "lnv")
                nc.scalar.mul(msq[:, :], sq_ps[:, :], 1.0 / E)
                mu2 = tl(wkp, [1, T], F32, "lnw")
                nc.vector.tensor_mul(mu2[:, :], nmu[:, :], nmu[:, :])
                nc.vector.tensor_sub(msq[:, :], msq[:, :], mu2[:, :])
                nc.scalar.activation(msq[:, :], msq[:, :], AF.Sqrt,
                                     bias=epst[:, :])
                rr = tl(wkp, [1, T], F32, "lnr")
                nc.vector.reciprocal_approx_fast(out=rr[:, :], in_=msq[:, :])
                rrb = tl(wkp, [1, T], BF16, "lnrb")
                nc.vector.tensor_copy(rrb[:, :], rr[:, :])
                rmub = tl(wkp, [1, T], BF16, "lnmb")
                nc.vector.tensor_mul(rmub[:, :], nmu[:, :], r32(rr[:, :]))
                rbh = tl(ps, [P, T], F32, t0)
                nc.tensor.matmul(rbh[:, :], onr[:, :], rrb[:, :],
                                 start=True, stop=True)
                mbh = tl(ps, [P, T], F32, t1)
                nc.tensor.matmul(mbh[:, :], onr[:, :], rmub[:, :],
                                 start=True, stop=True)
                for t in range(EB):
                    nc.vector.tensor_mul(dst[t][:, ssl], HT[t][:, ssl],
                                         rbh[:, :])
                    nc.vector.tensor_add(dst[t][:, ssl], dst[t][:, ssl],
                                         mbh[:, :])

            def v_gemm(l, s):
                """VT[s][tb] <- A @ Wv (token-major, no bias)."""
                for tbh in range(2):
                    tbs = (2 * tbh, 2 * tbh + 1)
                    for eh in range(c.NEH):
                        esl = slice(eh * c.EH, (eh + 1) * c.EH)
                        accs = [tl(ps, [P, c.EH], F32, f"g{tb % 2}")
                                for tb in tbs]
                        for k in range(EB):
                            wvs = tl(wkp, [P, c.EH], BF16, f"wv{k % 2}")
                            nc.sync.dma_start(out=wvs, in_=wv_d[l, k, :, esl])
                            for i, tb in enumerate(tbs):
                                nc.tensor.matmul(
                                    accs[i][:, :],
                                    AT[k][:, s * T + tb * P:s * T + (tb + 1) * P],
                                    wvs[:, :], start=(k == 0),
                                    stop=(k == EB - 1))
                        for i, tb in enumerate(tbs):
                            nc.vector.tensor_copy(VT[s][tb][:, esl],
                                                  accs[i][:, :])

            qkb = [None, None]

            def qk_bias(l):
                for wi, bd in enumerate((bq_d, bk_d)):
                    qkb[wi] = tl(wkp, [P, EB], F32, f"bias_qk{wi}")
                    nc.sync.dma_start(out=qkb[wi], in_=bd[l, :, :])

            def qk_eo(l, s, wi, eo):
                ssl = slice(s * T, (s + 1) * T)
                wd, dst = ((wq_d, QT), (wk_d, KT))[wi]
                wc = tl(wkp, [P, EB, P], BF16, f"wqk{(2 * eo + wi) % 3}")
                nc.sync.dma_start(out=wc, in_=wd[l, eo, :, :, :])
                acc = tl(ps, [P, T], F32, f"g{(2 * eo + wi) % 2}")
                for k in range(EB):
                    nc.tensor.matmul(acc[:, :], wc[:, k, :], AT[k][:, ssl],
                                     start=(k == 0), stop=(k == EB - 1))
                nc.vector.tensor_scalar_add(dst[eo][:, :], acc[:, :],
                                            qkb[wi][:, eo:eo + 1])

            def attn_eb(l, s, eb, dbg=False):
                """One 2-head block of causal attention for seq s:
                S^T = K Q^T (+causal bias) -> exp -> per-head sums (2-row PSUM
                group) -> P^T V (both heads packed in one PSUM bank via column
                tile positions, descending-b groups so the second head's
                start=True never clears an open first-head group) ->
                normalize+evict to OT in a single DVE mul."""
                opsB = tl(ps, [P, T], F32, "o0")
                sump2 = tl(ps, [2, T], F32, "su" if eb % 2 == 0 else "o1")
                for hp in range(HPB):
                    hh = eb * HPB + hp
                    r0 = hp * HD
                    qs = QT[eb][r0:r0 + HD, :]
                    ks = KT[eb][r0:r0 + HD, :]
                    for b in range(NB):
                        csl = slice(b * P, T)
                        dsl = slice(b * P, (b + 1) * P)
                        sps = tl(ps, [P, T], F32, f"s{b % 2}")
                        # causal mask folded in as an additive bias on the
                        # diagonal block (start=True writes the bias, the
                        # S matmul accumulates on it / overwrites beyond)
                        nc.tensor.matmul(sps[:, dsl], mbias[:, :],
                                         id128[:, :], start=True, stop=False)
                        nc.tensor.matmul(sps[:, csl],
                                         ks[:, b * P:(b + 1) * P],
                                         qs[:, csl], start=False, stop=True)
                        nc.scalar.activation(PT[b][:, csl], sps[:, csl],
                                             AF.Exp)
                        nc.tensor.matmul(sump2[:, csl],
                                         hsel[:, hh, 2 * eb:2 * eb + 2],
                                         PT[b][:, csl],
                                         start=(hp == 0 and b == 0),
                                         stop=(hp == HPB - 1 and b == NB - 1))
                    for j in range(NB):
                        jsl = slice(j * P, (j + 1) * P)
                        for b in range(j, -1, -1):
                            vs = VT[s][b][:, hh * HD:(hh + 1) * HD]
                            nc.tensor.matmul(opsB[r0:r0 + HD, jsl], vs,
                                             PT[b][:, jsl],
                                             start=(b == j), stop=(b == 0))
                    if dbg and eb == 0 and hp == 0:
                        dump(4, PT[0][:, :])
                sums2 = tl(wkp, [2, T], F32, "sums2")
                nc.scalar.copy(sums2[:, :], sump2[:, :])
                rr2 = tl(wkp, [2, T], F32, "rr2")
                nc.vector.reciprocal_approx_fast(out=rr2[:, :], in_=sums2[:, :])
                rrb2 = tl(wkp, [2, T], BF16, "rrb2")
                nc.vector.tensor_copy(rrb2[:, :], rr2[:, :])
                rb = tl(ps, [P, T], F32, "rb")
                nc.tensor.matmul(rb[:, :], hb2[:, :], rrb2[:, :],
                                 start=True, stop=True)
                rbs = tl(wkp, [P, T], F32, f"rbs{eb % 2}")
                nc.scalar.copy(rbs[:, :], rb[:, :])
                nc.vector.tensor_mul(OT[s][eb][:, :], opsB[:, :], rbs[:, :])

            obias = [None, None]

            def o_proj_chunk(l, s, eos):
                ssl = slice(s * T, (s + 1) * T)
                if obias[s] is None or eos[0] == 0:
                    obias[s] = tl(wkp, [P, EB], F32, f"bias_o{s}")
                    nc.sync.dma_start(out=obias[s], in_=bo_d[l, :, :])
                for eo in eos:
                    wc = tl(wkp, [P, EB, P], BF16, f"wo{eo % 2}")
                    nc.sync.dma_start(out=wc, in_=wo_d[l, eo, :, :, :])
                    acc = tl(ps, [P, T], F32, f"g{eo % 2}")
                    for k in range(EB):
                        nc.tensor.matmul(acc[:, :], wc[:, k, :], OT[s][k][:, :],
                                         start=(k == 0), stop=(k == EB - 1))
                    tmp = tl(wkp, [P, T], F32R, f"te{eo % 2}")
                    nc.vector.tensor_scalar_add(tmp[:, :], acc[:, :],
                                                obias[s][:, eo:eo + 1])
                    nc.vector.tensor_add(HT[eo][:, ssl], HT[eo][:, ssl],
                                         tmp[:, :])

            mlpb = [None, None]

            def mlp_bias(l, th):
                mlpb[0] = tl(wkp, [P, HIDB], F32, f"bias_1{th}")
                nc.sync.dma_start(out=mlpb[0], in_=b1_d[l, :, :])
                mlpb[1] = tl(wkp, [P, EB], F32, f"bias_2{th}")
                nc.sync.dma_start(out=mlpb[1], in_=b2_d[l, :, :])

            def fc1_chunk(l, th, hids):
                tsl = slice(th * T, (th + 1) * T)
                for hid in hids:
                    wc = tl(wkp, [P, EB, P], BF16, f"w1{hid % 2}")
                    nc.sync.dma_start(out=wc, in_=w1_d[l, hid, :, :, :])
                    acc = tl(ps, [P, T], F32, f"g{hid % 2}")
                    for k in range(EB):
                        nc.tensor.matmul(acc[:, :], wc[:, k, :], AT[k][:, tsl],
                                         start=(k == 0), stop=(k == EB - 1))
                    nc.scalar.activation(GT[hid][:, :], acc[:, :], AF.Gelu,
                                         bias=mlpb[0][:, hid:hid + 1])

            def fc2_half(l, th):
                tsl = slice(th * T, (th + 1) * T)
                b2t = mlpb[1]
                HB2 = HIDB // 2
                for eo in range(EB):
                    wch = [tl(wkp, [P, HB2, P], BF16, f"w2{hf}")
                           for hf in range(2)]
                    for hf in range(2):
                        nc.sync.dma_start(
                            out=wch[hf],
                            in_=w2_d[l, eo, :, hf * HB2:(hf + 1) * HB2, :])
                    acc = tl(ps, [P, T], F32, f"g{eo % 2}")
                    for k in range(HIDB):
                        nc.tensor.matmul(acc[:, :], wch[k // HB2][:, k % HB2, :],
                                         GT[k][:, :],
                                         start=(k == 0), stop=(k == HIDB - 1))
                    tmp = tl(wkp, [P, T], F32R, f"te{eo % 2}")
                    nc.vector.tensor_scalar_add(tmp[:, :], acc[:, :],
                                                b2t[:, eo:eo + 1])
                    nc.vector.tensor_add(HT[eo][:, tsl], HT[eo][:, tsl],
                                         tmp[:, :])

            # ---- embedding ----
            xt = tl(wkp, [c.DIN, TOK], F32R, "w10")
            nc.sync.dma_start(out=xt, in_=r32(xt_d[:, :]))
            we1 = tl(wkp, [c.DIN, E], F32R, "w11")
            nc.sync.dma_start(out=we1, in_=r32(we1_d[:, :]))
            be1 = tl(wkp, [P, EB], F32, "bias_e1")
            nc.sync.dma_start(out=be1, in_=be1_d[:, :])
            be2 = tl(wkp, [P, EB], F32, "bias_e2")
            nc.sync.dma_start(out=be2, in_=be2_d[:, :])
            for eo in range(EB):
                for h in range(2):
                    sl = slice(h * T, (h + 1) * T)
                    acc = tl(ps, [P, T], F32, f"g{(eo * 2 + h) % 2}")
                    nc.tensor.matmul(acc[:, :], we1[:, eo * P:(eo + 1) * P],
                                     xt[:, sl], start=True, stop=True)
                    nc.scalar.activation(AT[eo][:, sl], acc[:, :], AF.Identity,
                                         bias=be1[:, eo:eo + 1])
            for eo in range(EB):
                wc = tl(wkp, [P, EB, P], BF16, f"wqk{eo % 3}")
                nc.sync.dma_start(out=wc, in_=we2_d[eo, :, :, :])
                for h in range(2):
                    sl = slice(h * T, (h + 1) * T)
                    acc = tl(ps, [P, T], F32, f"s{(eo * 2 + h) % 2}")
                    for k in range(EB):
                        nc.tensor.matmul(acc[:, :], wc[:, k, :], AT[k][:, sl],
                                         start=(k == 0), stop=(k == EB - 1))
                    nc.scalar.activation(HT[eo][:, sl], acc[:, :], AF.Identity,
                                         bias=be2[:, eo:eo + 1])

            def dump(idx, ap, dt=BF16):
                """DMA a [rows<=P, T] tile to dbg[idx] (via f32 staging)."""
                if dbg_d is None:
                    return
                rows = ap.partition_size()
                stg = tl(wkp, [rows, T], F32, "dbgstg")
                nc.vector.tensor_copy(stg[:, :], ap)
                nc.sync.dma_start(out=dbg_d[idx, 0:rows, :], in_=stg[:, :])

            # ---- transformer layers ----
            # Interleaved emission: each attention block's chain stalls are
            # filled with independent GEMM chunks (next-seq Q/K or fc1) so the
            # PE instruction stream stays dense and HAM stays warm.
            for l in range(L):
                dbgl = debug and l == 0
                qk_bias(l)
                ln_half(0, AT, "s0", "s1")
                if dbgl:
                    dump(0, AT[0][:, 0:T])
                v_gemm(l, 0)
                for eo in range(EB):
                    qk_eo(l, 0, 0, eo)
                    qk_eo(l, 0, 1, eo)
                if dbgl:
                    dump(1, QT[0][:, :])
                    dump(2, KT[0][:, :])
                    dump(3, VT[0][0][:, 0:T])
                ln_half(1, AT, "s0", "s1")
                v_gemm(l, 1)
                for eb in range(EB):
                    attn_eb(l, 0, eb, dbg=dbgl)
                    qk_eo(l, 1, 0, eb)
                    qk_eo(l, 1, 1, eb)
                if dbgl:
                    dump(6, OT[0][0][:, :])
                o_proj_chunk(l, 0, range(EB))
                if dbgl:
                    dump(7, HT[0][:, 0:T])
                ln_half(0, AT, "g0", "g1")
                mlp_bias(l, 0)
                for eb in range(EB):
                    attn_eb(l, 1, eb)
                    fc1_chunk(l, 0, range(4 * eb, 4 * eb + 4))
                fc2_half(l, 0)
                o_proj_chunk(l, 1, range(EB))
                ln_half(1, AT, "s0", "s1")
                mlp_bias(l, 1)
                fc1_chunk(l, 1, range(HIDB))
                fc2_half(l, 1)

            # ---- final LN + head ----
            bht = tl(wkp, [P, EB], F32, "bias_h")
            nc.sync.dma_start(out=bht, in_=bh_d[:, :])
            ln_half(0, AT, "s0", "s1")
            ln_half(1, AT, "s0", "s1")
            for eo in range(EB):
                wc = tl(wkp, [P, EB, P], BF16, f"wqk{eo % 3}")
                nc.sync.dma_start(out=wc, in_=whd_d[eo, :, :, :])
                for h in range(2):
                    sl = slice(h * T, (h + 1) * T)
                    acc = tl(ps, [P, T], F32, f"g{(eo * 2 + h) % 2}")
                    for k in range(EB):
                        nc.tensor.matmul(acc[:, :], wc[:, k, :], AT[k][:, sl],
                                         start=(k == 0), stop=(k == EB - 1))
                    tmp = tl(wkp, [P, T], F32, f"th{h}")
                    nc.scalar.activation(tmp[:, :], acc[:, :], AF.Identity,
                                         bias=bht[:, eo:eo + 1])
                    nc.sync.dma_start(out=out_d[eo, :, sl], in_=tmp[:, :])

    nc.finalize()
    return nc


# --------------------------------------------------------------------------
# host-side weight preparation
# --------------------------------------------------------------------------

def _col_tile(w):
    """[Ein, Eout] -> [Eout/P, P(ein-in-blk), Ein/P, P(eout)]."""
    ein, eout = w.shape
    return np.ascontiguousarray(
        w.reshape(ein // P, P, eout // P, P).transpose(2, 1, 0, 3))


def _bias_t(b):
    """[E] -> [P, E/P]  (column per 128-block)."""
    return np.ascontiguousarray(b.reshape(-1, P).T)


def prep_global(cfg, inp):
    c = cfg
    f = np.float32
    bf = ml_dtypes.bfloat16
    g1 = np.asarray(inp["ln1_g"], f); c1 = np.asarray(inp["ln1_b"], f)
    g2 = np.asarray(inp["ln2_g"], f); c2 = np.asarray(inp["ln2_b"], f)
    sc = 1.0 / np.sqrt(c.HD)
    L = c.L
    wq = np.empty((L, c.EB, P, c.EB, P), bf)
    wkk = np.empty((L, c.EB, P, c.EB, P), bf)
    wv = np.empty((L, c.EB, P, c.E), bf)
    wo = np.empty((L, c.EB, P, c.EB, P), bf)
    w1 = np.empty((L, c.HIDB, P, c.EB, P), bf)
    w2 = np.empty((L, c.EB, P, c.HIDB, P), bf)
    bq = np.empty((L, P, c.EB), f); bk = np.empty((L, P, c.EB), f)
    bo = np.empty((L, P, c.EB), f)
    b1 = np.empty((L, P, c.HIDB), f); b2 = np.empty((L, P, c.EB), f)
    for l in range(L):
        Wq = np.asarray(inp["Wq"][l], f); Wk = np.asarray(inp["Wk"][l], f)
        Wv = np.asarray(inp["Wv"][l], f); Wo = np.asarray(inp["Wo"][l], f)
        W1 = np.asarray(inp["W1"][l], f); W2 = np.asarray(inp["W2"][l], f)
        wq[l] = _col_tile(Wq * (g1[l][:, None] * sc)).astype(bf)
        wkk[l] = _col_tile(Wk * g1[l][:, None]).astype(bf)
        wv[l] = (Wv * g1[l][:, None]).reshape(c.EB, P, c.E).astype(bf)
        wo[l] = _col_tile(Wo).astype(bf)
        w1[l] = _col_tile(W1 * g2[l][:, None]).astype(bf)
        w2[l] = _col_tile(W2).astype(bf)
        bq[l] = _bias_t((Wq.T @ c1[l] + np.asarray(inp["bq"][l], f)) * sc)
        bk[l] = _bias_t(Wk.T @ c1[l] + np.asarray(inp["bk"][l], f))
        # V bias (incl. LN-bias term) folded through Wo into the O-proj bias
        bveff = Wv.T @ c1[l] + np.asarray(inp["bv"][l], f)
        bo[l] = _bias_t(Wo.T @ bveff + np.asarray(inp["bo"][l], f))
        b1[l] = _bias_t(W1.T @ c2[l] + np.asarray(inp["b1"][l], f))
        b2[l] = _bias_t(np.asarray(inp["b2"][l], f))
    gf = np.asarray(inp["lnf_g"], f); cf = np.asarray(inp["lnf_b"], f)
    Whd = np.asarray(inp["Whead"], f)
    H = c.H
    hsel = np.zeros((P, H, H), bf)
    for h in range(H):
        hsel[:, h, h] = 1.0
    hb2 = np.zeros((c.HPB, P), f)
    for hp in range(c.HPB):
        hb2[hp, hp * c.HD:(hp + 1) * c.HD] = 1.0
    return dict(
        wemb1=np.ascontiguousarray(np.asarray(inp["Wemb1"], f)),
        wemb2col=_col_tile(np.asarray(inp["Wemb2"], f)).astype(bf),
        wheadcol=_col_tile(Whd * gf[:, None]).astype(bf),
        wqcol=wq, wkcol=wkk, wvrow=wv, wocol=wo, w1col=w1, w2col=w2,
        bq_t=bq, bk_t=bk, bo_t=bo, b1_t=b1, b2_t=b2,
        bemb1_t=_bias_t(np.asarray(inp["bemb1"], f)),
        bemb2_t=_bias_t(np.asarray(inp["bemb2"], f)),
        bhead_t=_bias_t(Whd.T @ cf),
        ones_col=np.ones((P, 1), np.float32),
        ones_row=np.ones((1, P), bf),
        hsel=hsel,
        hb2=hb2.astype(bf),
        mbias=np.triu(np.full((P, P), -30000.0), 1).astype(bf),
        id128=np.eye(P).astype(bf),
        epsv=np.full((1, 1), LN_EPS, np.float32),
    )


def make_in_maps(cfg, inp, n_cores=NCORES):
    c = cfg
    g = prep_global(cfg, inp)
    x = np.asarray(inp["x"], np.float32)          # [B, T, DIN]
    B = x.shape[0]
    spc = B // n_cores
    assert spc == c.NS
    maps = []
    for core in range(n_cores):
        xs = x[core * spc:(core + 1) * spc]       # [NS, T, DIN]
        xt = np.ascontiguousarray(xs.reshape(c.TOK, c.DIN).T)
        m = dict(g)
        m["xt"] = xt
        maps.append(m)
    return maps


def assemble_out(cfg, results, n_cores=NCORES):
    c = cfg
    B = n_cores * c.NS
    out = np.empty((B, c.T, c.E), np.float32)
    for core in range(n_cores):
        o = results[core]["out"].reshape(c.E, c.TOK)
        for s in range(c.NS):
            out[core * c.NS + s] = o[:, s * c.T:(s + 1) * c.T].T
    return out


_NC_CACHE = {}


def kernel(**inputs) -> np.ndarray:
    cfg = Cfg()
    assert int(inputs["n_head"]) == cfg.H
    if "full" not in _NC_CACHE:
        _NC_CACHE["full"] = build_nc(cfg)
    nc = _NC_CACHE["full"]
    in_maps = make_in_maps(cfg, inputs)
    res = run_bass_kernel_spmd(nc, in_maps, core_ids=list(range(NCORES)))
    return assemble_out(cfg, res.results)
